# revision 12
# baseline (speedup 1.0000x reference)
"""Trainium2 Bass kernel for CommunityGNN (2-layer GCN + contrastive loss scalar).

Math used (verified vs reference to ~2e-5 rel):
  - total = sum(z @ z.T) == ||sum_i z_i||^2  (no NxN matrix needed)
  - pos_sum = sum over UNIQUE edges (u,v) of z[u].z[v]  (mask dedups)
  - GCN layer: out = D^-1/2 (A+I) D^-1/2 (X W) + b, computed as
      pre-scale rows by dinv -> gather/segment-sum by dst -> post-scale by dinv.

Device strategy (8 NeuronCores, SPMD, one program):
  - Nodes row-sharded: core c owns rows [c*1500, (c+1)*1500), padded to 1536.
  - Transform (X@W) sharded per core; scaled rows all-gathered into a DRAM
    table [12288, 64]; per-edge messages fetched with indirect DMA (128 rows
    per call, dst-sharded edges sorted by dst) and segment-summed on the PE
    via 0/1 indicator matmuls accumulating in PSUM (32-node windows, 4
    windows col-tiled per [128,64] PSUM tile). pos_sum reuses the edge-pass
    aggregation on z: pos_sum = S_edge - sum||z||^2 - S_dup.
  - Final reductions (sum z rows, sum z*g3) via ones-vector matmuls; host does
    only the cross-core combine and the final scalar formula.

Host-side work is index manipulation only: sort by dst, duplicate detection
for the positive mask, CSR degree counts, index packing, layout transforms.
"""

import os
import sys
import time

import numpy as np

for _p in ("/opt/trn_rl_repo", os.path.expanduser("~/.axon_site/_ro/trn_rl_repo")):
    if _p not in sys.path and os.path.isdir(_p):
        sys.path.append(_p)

from concourse import bacc, bass, mybir, tile  # noqa: E402
from concourse.bass_utils import run_bass_kernel_spmd  # noqa: E402
from concourse.masks import make_identity  # noqa: E402

F32 = mybir.dt.float32
I32 = mybir.dt.int32

# Problem constants (hardcoded per harness contract).
N = 12000
E = 384000
F_IN = 128
H = 64
M = 8  # cores
P = 128  # partitions
WSZ = 32  # dst window size (one PE col-group)
WPG = 4  # windows per PSUM group tile

_stats = {}


# --------------------------------------------------------------------------
# Host preprocessing (indices only)
# --------------------------------------------------------------------------


def _plan_pass(src, dst, n, m):
    """Window/chunk plan for one gather+segment-sum pass, uniform across cores.

    src/dst: int arrays of edge endpoints (dst determines owner).
    Returns dict with idx (int16, gather-wrapped, [128, CE//16]), drel
    (f32 [128, NC]), cw (chunks per window), and layout constants.
    """
    npc = n // m  # nodes per core
    g = (npc + P - 1) // P  # PSUM groups per core
    nl = g * P  # padded local nodes
    nw = g * WPG  # windows per core (incl dummy tail windows)

    owner = dst // npc
    dloc = dst % npc
    win = dloc // WSZ

    # table row for a global node id (partition-major within owner block)
    def tabrow(node):
        o = node // npc
        l = node % npc
        return o * nl + (l % P) * g + l // P

    src_tab = tabrow(src)

    # sort by (owner, window, dloc) for stable per-window runs
    key = (owner * nw + win) * npc + dloc
    order = np.argsort(key, kind="stable")
    so = src_tab[order]
    do = dloc[order]
    wo = (owner * nw + win)[order]  # global window id 0..m*nw

    counts = np.bincount(wo, minlength=m * nw)
    cw = int((counts.max() + P - 1) // P)  # chunks per window, uniform
    ce_w = cw * P  # padded edges per window

    idx = np.zeros((m, nw, ce_w), dtype=np.int32)
    drel = np.full((m, nw, ce_w), -1.0, dtype=np.float32)

    # scatter edges into padded window slots
    offs = np.zeros(m * nw + 1, dtype=np.int64)
    np.cumsum(counts, out=offs[1:])
    pos_in_win = np.arange(len(so)) - offs[wo]
    ci = wo // nw  # core
    wi = wo % nw
    idx[ci, wi, pos_in_win] = so.astype(np.int32)
    drel[ci, wi, pos_in_win] = (do - (wi * WSZ)).astype(np.float32)

    nc_chunks = nw * cw  # chunks per core per pass
    ce = nc_chunks * P  # edges per core per pass (padded)

    # indirect-DMA layout: idx32[p, k] = table row of edge k*128+p
    idx_w = np.ascontiguousarray(
        idx.reshape(m, nc_chunks, P).transpose(0, 2, 1)
    ).astype(np.int32)  # [m, 128, nc_chunks]

    # drel per (partition=i%128, chunk=i//128)
    drel_w = np.ascontiguousarray(
        drel.reshape(m, nc_chunks, P).transpose(0, 2, 1)
    )  # [m, 128, nc_chunks]

    return dict(idx=idx_w, drel=drel_w, cw=cw, g=g, nl=nl, nw=nw, nch=nc_chunks)


def preprocess(x, edge_index, n=N, m=M):
    npc = n // m
    ei = np.asarray(edge_index)
    loop = np.arange(n, dtype=np.int64)
    src = np.concatenate([ei[0].astype(np.int64), loop])
    dst = np.concatenate([ei[1].astype(np.int64), loop])

    ep = _plan_pass(src, dst, n, m)

    # duplicate occurrences beyond the first of each (src,dst) pair in E
    pk = ei[0].astype(np.int64) * n + ei[1].astype(np.int64)
    o = np.argsort(pk, kind="stable")
    pks = pk[o]
    dupmask = np.zeros(len(pks), dtype=bool)
    dupmask[1:] = pks[1:] == pks[:-1]
    dsrc, ddst = pks[dupmask] // n, pks[dupmask] % n
    pos_cnt = len(pk) - int(dupmask.sum())
    if len(dsrc) == 0:
        dsrc = np.array([0], dtype=np.int64)
        ddst = np.array([0], dtype=np.int64)
        dup_w = 0.0
    else:
        dup_w = 1.0
    up = _plan_pass(dsrc, ddst, n, m)
    up["weight"] = dup_w

    deg = np.bincount(dst, minlength=n).astype(np.float32)
    g = ep["g"]
    degt = np.ones((m, P, g), dtype=np.float32)
    dl = np.arange(npc)
    for c in range(m):
        degt[c, dl % P, dl // P] = deg[c * npc : (c + 1) * npc]

    xT = np.asarray(x, dtype=np.float32)
    xTs = np.stack(
        [np.ascontiguousarray(xT[c * npc : (c + 1) * npc].T) for c in range(m)]
    )  # [m, F_IN, npc]

    iota = np.tile(np.arange(WSZ, dtype=np.float32), (P, 1))  # [128, 32]

    # mask for the last node-major group: 1 where local id < npc
    nl = ep["nl"]
    pad0 = npc - (ep["g"] - 1) * P  # valid partitions in last group
    mlast = (np.arange(P) < pad0).astype(np.float32)[:, None]

    return dict(ep=ep, up=up, pos_cnt=pos_cnt, degt=degt, xTs=xTs, iota=iota,
                mlast=mlast)


# --------------------------------------------------------------------------
# Bass program
# --------------------------------------------------------------------------


def build_program(n, m, cw_e, cw_u, debug_outs=False):
    npc = n // m
    g = (npc + P - 1) // P
    nl = g * P
    nw = g * WPG
    ntab = m * nl
    nch_e = nw * cw_e  # chunks per edge pass
    nch_u = nw * cw_u
    ce = nch_e * P
    cu = nch_u * P
    fh = g * H  # free size of node-major [128, g*64] tiles
    nch_max = max(nch_e, nch_u)

    nc = bacc.Bacc("TRN2", target_bir_lowering=False, debug=False, num_devices=m)

    # ---- I/O ----
    xT_d = nc.dram_tensor("xT", [F_IN, npc], F32, kind="ExternalInput")
    w1_d = nc.dram_tensor("w1", [F_IN, H], F32, kind="ExternalInput")
    w2_d = nc.dram_tensor("w2", [H, H], F32, kind="ExternalInput")
    b1_d = nc.dram_tensor("b1r", [P, H], F32, kind="ExternalInput")
    b2_d = nc.dram_tensor("b2r", [P, H], F32, kind="ExternalInput")
    degt_d = nc.dram_tensor("degt", [P, g], F32, kind="ExternalInput")
    iota_d = nc.dram_tensor("iota", [P, WSZ], F32, kind="ExternalInput")
    eidx_d = nc.dram_tensor("eidx", [P, nch_e], I32, kind="ExternalInput")
    edrel_d = nc.dram_tensor("edrel", [P, nch_e], F32, kind="ExternalInput")
    uidx_d = nc.dram_tensor("uidx", [P, nch_u], I32, kind="ExternalInput")
    udrel_d = nc.dram_tensor("udrel", [P, nch_u], F32, kind="ExternalInput")
    mlast_d = nc.dram_tensor("mlast", [P, 1], F32, kind="ExternalInput")
    out_d = nc.dram_tensor("out", [1, 4 * fh], F32, kind="ExternalOutput")
    if debug_outs:
        dbg_h = nc.dram_tensor("dbg_h", [P, fh], F32, kind="ExternalOutput")
        dbg_emb = nc.dram_tensor("dbg_emb", [P, fh], F32, kind="ExternalOutput")
        dbg_z = nc.dram_tensor("dbg_z", [P, fh], F32, kind="ExternalOutput")
        dbg_g3 = nc.dram_tensor("dbg_g3", [P, fh], F32, kind="ExternalOutput")

    rg = [list(range(m))]

    with tile.TileContext(nc) as tc:
        import contextlib

        ctx = contextlib.ExitStack()
        with ctx:
            sb = ctx.enter_context(tc.tile_pool(name="sb", bufs=1))
            sb2 = ctx.enter_context(tc.tile_pool(name="sb2", bufs=2))
            gp = ctx.enter_context(tc.tile_pool(name="gp", bufs=3))
            pp = ctx.enter_context(tc.tile_pool(name="pp", bufs=2, space="PSUM"))
            pt_pool = pp
            dr = ctx.enter_context(tc.tile_pool(name="dr", bufs=1, space="DRAM"))

            # ---- resident SBUF constants ----
            xT_s = sb.tile([F_IN, nl], F32)
            if npc < nl:
                nc.vector.memset(xT_s[:, npc:nl], 0.0)
            nc.gpsimd.dma_start(out=xT_s[:, :npc], in_=xT_d[:])
            w1_s = sb.tile([F_IN, H], F32)
            nc.gpsimd.dma_start(out=w1_s[:], in_=w1_d[:])
            w2_s = sb.tile([H, H], F32)
            nc.gpsimd.dma_start(out=w2_s[:], in_=w2_d[:])
            b1_s = sb.tile([P, H], F32)
            nc.gpsimd.dma_start(out=b1_s[:], in_=b1_d[:])
            b2_s = sb.tile([P, H], F32)
            nc.gpsimd.dma_start(out=b2_s[:], in_=b2_d[:])
            degt_s = sb.tile([P, g], F32)
            nc.gpsimd.dma_start(out=degt_s[:], in_=degt_d[:])
            iota_s = sb.tile([P, WSZ], F32)
            nc.gpsimd.dma_start(out=iota_s[:], in_=iota_d[:])
            eidx_s = sb.tile([P, nch_e], I32)
            nc.gpsimd.dma_start(out=eidx_s[:], in_=eidx_d[:])
            edrel_s = sb.tile([P, nch_e], F32)
            nc.gpsimd.dma_start(out=edrel_s[:], in_=edrel_d[:])
            uidx_s = sb.tile([P, nch_u], I32)
            nc.gpsimd.dma_start(out=uidx_s[:], in_=uidx_d[:])
            udrel_s = sb.tile([P, nch_u], F32)
            nc.gpsimd.dma_start(out=udrel_s[:], in_=udrel_d[:])
            mlast_s = sb.tile([P, 1], F32)
            nc.gpsimd.dma_start(out=mlast_s[:], in_=mlast_d[:])
            id_s = sb.tile([P, P], F32)
            make_identity(nc, id_s[:])
            ones_s = sb.tile([P, 1], F32)
            nc.vector.memset(ones_s[:], 1.0)

            # ---- dinv = rsqrt(deg), Newton-refined ----
            dinv_s = sb.tile([P, g], F32)
            rec = sb.tile([P, g], F32)
            nc.vector.reciprocal(rec[:], degt_s[:])
            nc.scalar.activation(dinv_s[:], rec[:], mybir.ActivationFunctionType.Sqrt)
            t1 = sb.tile([P, g], F32)
            nc.vector.tensor_tensor(
                out=t1[:], in0=dinv_s[:], in1=dinv_s[:], op=mybir.AluOpType.mult
            )
            nc.vector.tensor_tensor(
                out=t1[:], in0=t1[:], in1=degt_s[:], op=mybir.AluOpType.mult
            )
            nc.vector.tensor_scalar(
                out=t1[:], in0=t1[:], scalar1=-0.5, scalar2=1.5,
                op0=mybir.AluOpType.mult, op1=mybir.AluOpType.add,
            )
            nc.vector.tensor_tensor(
                out=dinv_s[:], in0=dinv_s[:], in1=t1[:], op=mybir.AluOpType.mult
            )

            # ---- indicator matrix build (edges pass; reused for L1+L2) ----
            ind_pool = ctx.enter_context(tc.tile_pool(name="ind", bufs=1))
            ind_e = ind_pool.tile([P, nch_max * WSZ], F32, tag="ind")
            nc.vector.tensor_tensor(
                out=ind_e[:, : nch_e * WSZ].rearrange("p (c w) -> p c w", w=WSZ),
                in0=iota_s[:, None, :].to_broadcast([P, nch_e, WSZ]),
                in1=edrel_s[:, :, None].to_broadcast([P, nch_e, WSZ]),
                op=mybir.AluOpType.is_equal,
            )

            # ---- helpers ----
            def transform_to_table(src_nm, w_s, kdim, tab_s, scale):
                """src feat-major [kdim, nl] @ w -> node-major table tile.

                src_nm: SBUF [kdim, nl] feat-major input (already includes
                dummy zero cols); w_s: [kdim, H] weights; writes tab_s [P, fh]
                node-major, rows scaled by `scale` [P, g] per-partition slices.
                """
                gT = sb2.tile([H, nl], F32, tag="gT")
                nsplit = [min(512, nl - j * 512) for j in range((nl + 511) // 512)]
                for j, w in enumerate(nsplit):
                    mm = pp.tile([H, 512], F32, tag="mmT")
                    nc.tensor.matmul(
                        out=mm[:, :w],
                        lhsT=w_s[:],
                        rhs=src_nm[:, j * 512 : j * 512 + w],
                        start=True, stop=True,
                    )
                    nc.vector.tensor_copy(out=gT[:, j * 512 : j * 512 + w], in_=mm[:, :w])
                for t in range(g):
                    trp = pp.tile([P, H], F32, tag="trp")
                    nc.tensor.transpose(
                        trp[:], gT[:, t * P : (t + 1) * P], id_s[:H, :H]
                    )
                    nc.vector.tensor_scalar(
                        out=tab_s[:, t * H : (t + 1) * H], in0=trp[:],
                        scalar1=scale[:, t : t + 1], scalar2=None,
                        op0=mybir.AluOpType.mult,
                    )

            def aggregate(tab_full, idx_s, ind, cw, consume):
                """Gather (128 rows per indirect DMA) + windowed segment-sum."""
                for t in range(g):
                    base = t * WPG * cw
                    gb = gp.tile([P, WPG * cw, H], F32, tag="gb")
                    for j in range(WPG * cw):
                        nc.gpsimd.indirect_dma_start(
                            out=gb[:, j, :], out_offset=None, in_=tab_full[:],
                            in_offset=bass.IndirectOffsetOnAxis(
                                ap=idx_s[:, base + j : base + j + 1], axis=0
                            ),
                        )
                    pt = pt_pool.tile([P, H], F32, tag="pt")
                    for w in range(WPG):
                        for k in range(cw):
                            ci = base + w * cw + k
                            nc.tensor.matmul(
                                out=pt[w * WSZ : (w + 1) * WSZ, :],
                                lhsT=ind[:, ci * WSZ : (ci + 1) * WSZ],
                                rhs=gb[:, w * cw + k, :],
                                start=(k == 0), stop=(k == cw - 1),
                                tile_position=(0, w * WSZ),
                            )
                    consume(t, pt)

            def allgather(tab_s, name):
                bounce = dr.tile([nl, H], F32, name=f"bn_{name}")
                full = dr.tile([ntab, H], F32, addr_space="Shared", name=f"tf_{name}")
                nc.gpsimd.dma_start(
                    out=bounce[:].rearrange("(p t) f -> p t f", p=P),
                    in_=tab_s[:].rearrange("p (t f) -> p t f", f=H),
                )
                nc.gpsimd.collective_compute(
                    "AllGather",
                    mybir.AluOpType.bypass,
                    ins=[bounce.opt()],
                    outs=[full.opt()],
                    replica_groups=rg,
                )
                return full

            # ---- layer 1 ----
            tab1_s = sb2.tile([P, fh], F32, tag="tab")
            transform_to_table(xT_s, w1_s, F_IN, tab1_s, dinv_s)
            # xT has npc cols; nl-npc dummy cols need zeros in table:
            # handled below by zeroing the dummy region of tab1_s.
            tab1_full = allgather(tab1_s, "t1")

            h_s = sb2.tile([P, fh], F32, tag="h")

            def consume_l1(t, pt):
                sl = h_s[:, t * H : (t + 1) * H]
                nc.vector.tensor_scalar(
                    out=sl, in0=pt[:], scalar1=dinv_s[:, t : t + 1], scalar2=None,
                    op0=mybir.AluOpType.mult,
                )
                nc.vector.tensor_tensor(out=sl, in0=sl, in1=b1_s[:], op=mybir.AluOpType.add)
                nc.vector.tensor_scalar(
                    out=sl, in0=sl, scalar1=0.0, scalar2=None, op0=mybir.AluOpType.max
                )

            aggregate(tab1_full, eidx_s, ind_e, cw_e, consume_l1)
            if debug_outs:
                nc.gpsimd.dma_start(out=dbg_h[:], in_=h_s[:])

            # ---- layer 2 ----
            hT_s = sb2.tile([H, nl], F32, tag="hT")
            for t in range(g):
                trp2 = pp.tile([H, P], F32, tag="trp")
                nc.tensor.transpose(trp2[:], h_s[:, t * H : (t + 1) * H], id_s[:])
                nc.vector.tensor_copy(out=hT_s[:, t * P : (t + 1) * P], in_=trp2[:])
            tab2_s = sb2.tile([P, fh], F32, tag="tab2")
            transform_to_table(hT_s, w2_s, H, tab2_s, dinv_s)
            tab2_full = allgather(tab2_s, "t2")

            emb_s = sb2.tile([P, fh], F32, tag="emb")

            def consume_l2(t, pt):
                sl = emb_s[:, t * H : (t + 1) * H]
                nc.vector.tensor_scalar(
                    out=sl, in0=pt[:], scalar1=dinv_s[:, t : t + 1], scalar2=None,
                    op0=mybir.AluOpType.mult,
                )
                nc.vector.tensor_tensor(out=sl, in0=sl, in1=b2_s[:], op=mybir.AluOpType.add)

            aggregate(tab2_full, eidx_s, ind_e, cw_e, consume_l2)
            if debug_outs:
                nc.gpsimd.dma_start(out=dbg_emb[:], in_=emb_s[:])

            # ---- normalize: z = emb * rsqrt(max(rownorm2, tiny)) ----
            sq = sb2.tile([P, fh], F32, tag="sq")
            nc.vector.tensor_tensor(out=sq[:], in0=emb_s[:], in1=emb_s[:], op=mybir.AluOpType.mult)
            s2 = sb.tile([P, g], F32)
            nc.vector.reduce_sum(
                out=s2[:],
                in_=sq[:].rearrange("p (t f) -> p t f", f=H),
                axis=mybir.AxisListType.X,
            )
            nc.vector.tensor_scalar(
                out=s2[:], in0=s2[:], scalar1=1e-24, scalar2=None, op0=mybir.AluOpType.max
            )
            rs = sb.tile([P, g], F32)
            rcp = sb.tile([P, g], F32)
            nc.vector.reciprocal(rcp[:], s2[:])
            nc.scalar.activation(rs[:], rcp[:], mybir.ActivationFunctionType.Sqrt)
            nt = sb.tile([P, g], F32)
            nc.vector.tensor_tensor(out=nt[:], in0=rs[:], in1=rs[:], op=mybir.AluOpType.mult)
            nc.vector.tensor_tensor(out=nt[:], in0=nt[:], in1=s2[:], op=mybir.AluOpType.mult)
            nc.vector.tensor_scalar(
                out=nt[:], in0=nt[:], scalar1=-0.5, scalar2=1.5,
                op0=mybir.AluOpType.mult, op1=mybir.AluOpType.add,
            )
            nc.vector.tensor_tensor(out=rs[:], in0=rs[:], in1=nt[:], op=mybir.AluOpType.mult)
            # zero dummy rows (local ids >= npc) by masking the scale
            nc.vector.tensor_tensor(
                out=rs[:, g - 1 : g], in0=rs[:, g - 1 : g], in1=mlast_s[:],
                op=mybir.AluOpType.mult,
            )

            z_s = sb2.tile([P, fh], F32, tag="z")
            nc.vector.tensor_tensor(
                out=z_s[:].rearrange("p (t f) -> p t f", f=H),
                in0=emb_s[:].rearrange("p (t f) -> p t f", f=H),
                in1=rs[:, :, None].to_broadcast([P, g, H]),
                op=mybir.AluOpType.mult,
            )
            if debug_outs:
                nc.gpsimd.dma_start(out=dbg_z[:], in_=z_s[:])

            z_full = allgather(z_s, "z")

            # ---- pass 3: edge-set aggregation of z (reuses ind_e/eidx) ----
            g3_s = sb2.tile([P, fh], F32, tag="g3")

            def consume_u(t, pt):
                nc.vector.tensor_copy(out=g3_s[:, t * H : (t + 1) * H], in_=pt[:])

            aggregate(z_full, eidx_s, ind_e, cw_e, consume_u)
            if debug_outs:
                nc.gpsimd.dma_start(out=dbg_g3[:], in_=g3_s[:])

            zg = sb2.tile([P, fh], F32, tag="zg")
            nc.vector.tensor_tensor(out=zg[:], in0=z_s[:], in1=g3_s[:], op=mybir.AluOpType.mult)

            # ---- pass 4: duplicate-edge mini pass over z ----
            ind_u = ind_pool.tile([P, nch_max * WSZ], F32, tag="ind")
            nc.vector.tensor_tensor(
                out=ind_u[:, : nch_u * WSZ].rearrange("p (c w) -> p c w", w=WSZ),
                in0=iota_s[:, None, :].to_broadcast([P, nch_u, WSZ]),
                in1=udrel_s[:, :, None].to_broadcast([P, nch_u, WSZ]),
                op=mybir.AluOpType.is_equal,
            )
            g3d_s = sb2.tile([P, fh], F32, tag="g3d")

            def consume_d(t, pt):
                nc.vector.tensor_copy(out=g3d_s[:, t * H : (t + 1) * H], in_=pt[:])

            aggregate(z_full, uidx_s, ind_u, cw_u, consume_d)
            zgd = sb2.tile([P, fh], F32, tag="zgd")
            nc.vector.tensor_tensor(out=zgd[:], in0=z_s[:], in1=g3d_s[:], op=mybir.AluOpType.mult)
            zz = sb2.tile([P, fh], F32, tag="zz")
            nc.vector.tensor_tensor(out=zz[:], in0=z_s[:], in1=z_s[:], op=mybir.AluOpType.mult)

            # ---- partial sums out: colsums of z, z*g3e, z*z, z*g3d ----
            out_s = sb.tile([1, 4 * fh], F32)
            for row, src in ((0, z_s), (1, zg), (2, zz), (3, zgd)):
                for j in range((fh + 511) // 512):
                    w = min(512, fh - j * 512)
                    ps = pp.tile([1, 512], F32, tag="ps")
                    nc.tensor.matmul(
                        out=ps[:, :w], lhsT=ones_s[:], rhs=src[:, j * 512 : j * 512 + w],
                        start=True, stop=True,
                    )
                    nc.vector.tensor_copy(
                        out=out_s[0:1, row * fh + j * 512 : row * fh + j * 512 + w],
                        in_=ps[:, :w],
                    )
            nc.gpsimd.dma_start(out=out_d[:], in_=out_s[:])

    nc.compile()
    return nc


# --------------------------------------------------------------------------
# Entry point
# --------------------------------------------------------------------------


def make_in_maps(x, edge_index, W1, b1, W2, b2, n=N, m=M):
    pre = preprocess(x, edge_index, n=n, m=m)
    ep, up = pre["ep"], pre["up"]
    in_maps = []
    for c in range(m):
        in_maps.append(
            {
                "xT": pre["xTs"][c],
                "w1": np.asarray(W1, np.float32),
                "w2": np.asarray(W2, np.float32),
                "b1r": np.tile(np.asarray(b1, np.float32)[None, :], (P, 1)),
                "b2r": np.tile(np.asarray(b2, np.float32)[None, :], (P, 1)),
                "degt": pre["degt"][c],
                "iota": pre["iota"],
                "eidx": ep["idx"][c],
                "edrel": ep["drel"][c],
                "uidx": up["idx"][c],
                "udrel": up["drel"][c],
                "mlast": pre["mlast"],
            }
        )
    return in_maps, pre


def combine(results, pos_cnt, dup_w, n=N):
    """Host combine of per-core partial sums -> final scalar.

    pos_sum = S_edgepass - sum||z||^2 (removes self-loops) - S_dup (removes
    duplicate-edge double counts).
    """
    s = np.zeros(H, dtype=np.float64)
    s_edge = 0.0
    zn2 = 0.0
    s_dup = 0.0
    for r in results:
        o = np.asarray(r["out"], dtype=np.float64).reshape(4, -1)
        s += o[0].reshape(-1, H).sum(axis=0)
        s_edge += o[1].sum()
        zn2 += o[2].sum()
        s_dup += o[3].sum()
    pos_sum = s_edge - zn2 - dup_w * s_dup
    total = float(s @ s)
    n2 = float(n) * float(n)
    ans = -pos_sum / pos_cnt + (total - pos_sum) / (n2 - pos_cnt)
    return np.float32(ans)


def kernel(x, edge_index, W1, b1, W2, b2):
    x = np.asarray(x)
    edge_index = np.asarray(edge_index)
    in_maps, pre = make_in_maps(x, edge_index, W1, b1, W2, b2)
    cw_e, cw_u = pre["ep"]["cw"], pre["up"]["cw"]
    nc = build_program(N, M, cw_e, cw_u, debug_outs=False)
    trace = bool(int(os.environ.get("GNN_TRACE", "0")))
    t0 = time.perf_counter()
    res = run_bass_kernel_spmd(nc, in_maps, list(range(M)), trace=trace)
    _stats["run_wall_s"] = time.perf_counter() - t0
    _stats["exec_time_ns"] = res.exec_time_ns
    _stats["profile_json"] = res.profile_json
    return np.asarray(
        combine(res.results, pre["pos_cnt"], pre["up"]["weight"]), dtype=np.float32
    )
